# revision 80
# baseline (speedup 1.0000x reference)
"""ESIM-style bidirectional cross-attention (LocalInterface) Bass kernel for TRN2.

Full inputs: px [32,512,512] f32, hx [32,512,512] f32, p_mask/h_mask [32,512] bool.
Data-parallel over batch: 8 NeuronCores x 4 batches each. Returns (m_p, m_h),
each [32,512,2048] f32.

v2 design (all-bf16 datapath; measured rel err 8.9e-3 vs the 2e-2 budget):

  The host ships px/hx in BOTH layouts as bf16 ([p,d] natural and [d,p]
  transposed, one stacked DRAM tensor, four 0.5MB DMAs/batch), so the
  device does ZERO input transposes: the e-matmul consumes the
  transposed copies directly and the value matmuls / output elementwise
  consume the natural copies. This removes 32 PE transposes and 8
  Act-engine PSUM evictions per batch vs v1, cutting the Act engine
  (the v1 bottleneck: 24 x ~720ns instructions/batch) down to the 8
  exps only it can do plus a balanced share of the evictions.

  e = px @ hx^T               bf16 matmuls, f32 PSUM accumulation
  u_aT[p,h] = exp(e + bias_p) Act exp during PSUM eviction; bias folds the
  u_bT[h,p] = exp(e^T+bias_h) constant shift -90 and the mask (-30090 ->
                              exp == 0) per partition, as in v1. u in bf16.
  e^T via PE f32r transpose (e must stay f32: |e| ~ 90, bf16 step ~0.5
  there would distort exp by ~60%; recomputing e^T as a second matmul
  pass measured WORSE on HW -- the 16 extra N=512 matmuls cost more
  than 16 transposes). The eT+exp stream is placed BETWEEN the two
  value-matmul groups so direction a's matmuls cover the e-eviction
  latency (sim: DMA engines 97.5% occupied with this order).
  px_hat = (u_bT^T @ hx) / s_b   value matmuls in bf16; s via 2-wide
  hx_hat = (u_aT^T @ px) / s_a   bf16 ones-column matmuls sharing the
                                 value matmul's stationary operand.
  Eviction work is balanced across the two capable engines: e copies
  split DVE/Act, direction-a hat scale on Act (Copy + per-partition
  scale AP), direction-b on DVE (tensor_scalar_mul), both writing
  straight into segment 0 of the output tile; DVE sub/mul fill
  segments 1/2 per 2-block half, and each half ships immediately
  (4 x 0.78MB output DMAs/batch) so the store overlaps compute.
  gpsimd does no elementwise work (its TensorTensor is 3.6x slower
  than DVE's) -- it only runs the mh output DMA ring + bias loads.
  The host prepends the verbatim px/hx segment and upcasts to f32
  after the gather.

Not adopted (measured): fp8-e4m3 prod segment (rel err 1.50e-2, no HW
speedup -- DVE fp8 stores + extra DMAs eat the byte savings); on-chip
pxT derivation (PE transposes lengthen the critical chain); host-
swizzled partition-contiguous DRAM layouts (defeats balance_dma_aps'
engine spray, 11us slower); merged 2MB input DMAs (serialize the SDMA
engines).

Per-core steady state (cost model): DMA 20.6MB -> 58.3us (the wall,
97.5% occupied), PE ~47.6us (96 matmuls/batch), Act/DVE ~30us each,
gpsimd ~9us. Measured on HW: 46-49us/exec min observed (M(16)/16),
~60-90us under tunnel contention.
"""

import numpy as np

NB = 4          # batches per core
NCORES = 8
S = 512         # P = H = D = 512
NBLK = 4        # 512 / 128
SHIFT = 90.0    # constant softmax shift (e ~ N(0, 512); see v1 analysis)
MASK_BIAS = -30090.0  # -SHIFT - 30000: exp underflows to exactly 0.0

_CACHED = {}


def _build(reps: int = 1):
    """Build the per-core Bass program.

    reps > 1 unrolls the whole per-core computation that many times
    (same inputs, same outputs) inside one NEFF; test.py uses this to
    measure steady-state per-execution time by differencing. The
    graded kernel() path always uses reps=1.
    """
    import concourse.tile as tile
    import concourse.mybir as mybir
    from concourse import bacc
    from concourse.masks import make_identity

    F32 = mybir.dt.float32
    F32R = mybir.dt.float32r
    BF16 = mybir.dt.bfloat16
    F8 = mybir.dt.float8e4
    EXP = mybir.ActivationFunctionType.Exp
    COPY = mybir.ActivationFunctionType.Copy

    nc = bacc.Bacc(None, target_bir_lowering=False)
    # stacked input tensor: [batch, tensor(px|hx|pxT|hxT), row, col]
    in_d = nc.dram_tensor("inp", [NB, 4, S, S], BF16, kind="ExternalInput")
    # exp biases, host-precomputed: [r, b, j] = -SHIFT if kept else MASK_BIAS
    bh_d = nc.dram_tensor("bh", [128, NB, NBLK], F32, kind="ExternalInput")
    bp_d = nc.dram_tensor("bp", [128, NB, NBLK], F32, kind="ExternalInput")
    mp_d = nc.dram_tensor("mp", [NB, S, 3 * S], BF16, kind="ExternalOutput")
    mh_d = nc.dram_tensor("mh", [NB, S, 3 * S], BF16, kind="ExternalOutput")

    with tile.TileContext(nc) as tc:
        with (
            tc.tile_pool(name="const", bufs=1) as const,
            tc.tile_pool(name="sbL", bufs=4) as sbL,
            tc.tile_pool(name="sbE", bufs=12) as sbE,
            tc.tile_pool(name="sbU", bufs=3) as sbU,
            tc.tile_pool(name="sbS", bufs=3) as sbS,
            tc.tile_pool(name="sbO", bufs=3) as sbO,
            tc.tile_pool(name="pe_p", bufs=2, space="PSUM") as pe_p,
            tc.tile_pool(name="pet_p", bufs=2, space="PSUM") as pet_p,
            tc.tile_pool(name="pv_p", bufs=3, space="PSUM") as pv_p,
            tc.tile_pool(name="ps_p", bufs=1, space="PSUM") as ps_p,
        ):
            ident = const.tile([128, 128], F32)
            make_identity(nc, ident)
            identr = const.tile([128, 128], F32R)
            nc.vector.tensor_copy(out=identr, in_=ident)
            # bf16 matmuls accept a 2-wide ones column for the s sums
            # (>=2-element contiguous PSUM dst requirement)
            ones_col = const.tile([128, 2], BF16)
            nc.vector.memset(ones_col, 1.0)
            # per-partition exp biases for every batch: one contiguous load
            bias_h = const.tile([128, NB, NBLK], F32)
            bias_p = const.tile([128, NB, NBLK], F32)
            nc.gpsimd.dma_start(out=bias_h, in_=bh_d[:, :, :])
            nc.gpsimd.dma_start(out=bias_p, in_=bp_d[:, :, :])

            for rep in range(reps):
                for b in range(NB):
                    # ---- loads: pxT/hxT first (feed the e-matmul), then
                    # the natural layouts (value matmuls / outputs) ----
                    ldT = sbL.tile([128, 2, NBLK, S], BF16, tag="ldT")
                    ldN = sbL.tile([128, 2, NBLK, S], BF16, tag="ldN")
                    if rep == 0 and b == 0:
                        # first batch: halved loads let the e-matmuls start
                        # on the first half ~1us earlier (one-shot ramp)
                        for t, eng in ((2, nc.sync), (3, nc.scalar)):
                            for hj in (0, 2):
                                eng.dma_start(
                                    out=ldT[:, t - 2, hj:hj + 2],
                                    in_=in_d[b, t, 128 * hj:128 * (hj + 2)]
                                    .rearrange("(i r) d -> r i d", r=128))
                    else:
                        nc.sync.dma_start(
                            out=ldT[:, 0],
                            in_=in_d[b, 2].rearrange("(i r) d -> r i d", r=128))
                        nc.scalar.dma_start(
                            out=ldT[:, 1],
                            in_=in_d[b, 3].rearrange("(i r) d -> r i d", r=128))
                    nc.sync.dma_start(
                        out=ldN[:, 0],
                        in_=in_d[b, 0].rearrange("(i r) d -> r i d", r=128))
                    nc.scalar.dma_start(
                        out=ldN[:, 1],
                        in_=in_d[b, 1].rearrange("(i r) d -> r i d", r=128))
                    px_t, hx_t = ldN[:, 0], ldN[:, 1]
                    pxT, hxT = ldT[:, 0], ldT[:, 1]

                    # ---- e = px @ hx^T [P,H]; u_aT = exp(e + bias_p) ----
                    # e PSUM->SBUF evictions split between DVE and Act.
                    e_sb = [sbE.tile([128, S], F32R, tag="e_sb",
                                     name=f"e_sb{rep}_{b}_{i}") for i in range(NBLK)]
                    u_aT = sbU.tile([128, NBLK, S], BF16, tag="u_aT")
                    for i in range(NBLK):
                        pe = pe_p.tile([128, S], F32, tag="pe")
                        for j in range(NBLK):
                            nc.tensor.matmul(
                                pe, pxT[:, j, 128 * i:128 * (i + 1)], hxT[:, j],
                                start=(j == 0), stop=(j == NBLK - 1),
                            )
                        nc.scalar.activation(
                            out=u_aT[:, i], in_=pe, func=EXP,
                            bias=bias_p[:, b, i:i + 1],
                        )
                        if i < 2:
                            nc.vector.tensor_copy(out=e_sb[i], in_=pe)
                        else:
                            nc.scalar.copy(out=e_sb[i], in_=pe)

                    r_t = sbS.tile([128, 2 * NBLK], F32, tag="r_t")

                    # ---- direction a (hx_hat, m_h): needs only u_aT ----
                    # hat eviction+scale on Act (Copy, per-partition scale);
                    # output assembled and shipped in 2-block halves so the
                    # DMA overlaps the remaining blocks' compute.
                    s_a = ps_p.tile([128, 2 * NBLK], F32, tag="sps")
                    mhb = sbO.tile([128, NBLK, 3, S], BF16, tag="mh_blk")
                    for j in range(NBLK):
                        pv = pv_p.tile([128, S], F32, tag="pv")
                        for i in range(NBLK):
                            nc.tensor.matmul(
                                pv, u_aT[:, i, 128 * j:128 * (j + 1)], px_t[:, i],
                                start=(i == 0), stop=(i == NBLK - 1),
                            )
                            nc.tensor.matmul(
                                s_a[:, 2 * j:2 * j + 2],
                                u_aT[:, i, 128 * j:128 * (j + 1)],
                                ones_col,
                                start=(i == 0), stop=(i == NBLK - 1),
                                skip_group_check=True,
                            )
                        nc.vector.reciprocal(
                            out=r_t[:, j:j + 1], in_=s_a[:, 2 * j:2 * j + 1])
                        nc.scalar.activation(
                            out=mhb[:, j, 0], in_=pv, func=COPY,
                            scale=r_t[:, j:j + 1])
                        if j % 2 == 1:
                            h = slice(j - 1, j + 1)
                            nc.vector.tensor_sub(
                                mhb[:, h, 1], hx_t[:, h], mhb[:, h, 0])
                            nc.vector.tensor_mul(
                                mhb[:, h, 2], hx_t[:, h], mhb[:, h, 0])
                            nc.gpsimd.dma_start(
                                out=mh_d[b, 128 * (j - 1):128 * (j + 1)]
                                .rearrange("(j r) s -> r j s", r=128),
                                in_=mhb[:, h].rearrange("r j f s -> r j (f s)"),
                            )

                    # ---- eT stream: PE f32r transpose of e (overlapped by
                    # direction a's value matmuls), exp -> u_bT ----
                    u_bT = sbU.tile([128, NBLK, S], BF16, tag="u_bT")
                    for j in range(NBLK):
                        pet = pet_p.tile([128, S], F32R, tag="pet")
                        for i in range(NBLK):
                            nc.tensor.matmul(
                                pet[:, 128 * i:128 * (i + 1)],
                                e_sb[i][:, 128 * j:128 * (j + 1)],
                                identr,
                                is_transpose=True,
                                start=(i == 0), stop=(i == NBLK - 1),
                                skip_group_check=True,
                            )
                        nc.scalar.activation(
                            out=u_bT[:, j], in_=pet, func=EXP,
                            bias=bias_h[:, b, j:j + 1],
                        )

                    # ---- direction b (px_hat, m_p): needs u_bT ----
                    # hat eviction+scale on DVE (tensor_scalar_mul) to balance.
                    s_b = ps_p.tile([128, 2 * NBLK], F32, tag="sps")
                    mpb = sbO.tile([128, NBLK, 3, S], BF16, tag="mp_blk")
                    for i in range(NBLK):
                        pv = pv_p.tile([128, S], F32, tag="pv")
                        for j in range(NBLK):
                            nc.tensor.matmul(
                                pv, u_bT[:, j, 128 * i:128 * (i + 1)], hx_t[:, j],
                                start=(j == 0), stop=(j == NBLK - 1),
                            )
                            nc.tensor.matmul(
                                s_b[:, 2 * i:2 * i + 2],
                                u_bT[:, j, 128 * i:128 * (i + 1)],
                                ones_col,
                                start=(j == 0), stop=(j == NBLK - 1),
                                skip_group_check=True,
                            )
                        nc.vector.reciprocal(
                            out=r_t[:, NBLK + i:NBLK + i + 1],
                            in_=s_b[:, 2 * i:2 * i + 1])
                        nc.vector.tensor_scalar_mul(
                            out=mpb[:, i, 0], in0=pv,
                            scalar1=r_t[:, NBLK + i:NBLK + i + 1])
                        fin = (rep == reps - 1 and b == NB - 1)
                        if fin and i >= 2:
                            # tail: per-block assembly + DMA on alternating
                            # HWDGE rings so the last store is only 0.39MB
                            h = slice(i, i + 1)
                            nc.vector.tensor_sub(
                                mpb[:, h, 1], px_t[:, h], mpb[:, h, 0])
                            nc.vector.tensor_mul(
                                mpb[:, h, 2], px_t[:, h], mpb[:, h, 0])
                            eng = nc.sync if i == 2 else nc.scalar
                            eng.dma_start(
                                out=mp_d[b, 128 * i:128 * (i + 1)]
                                .rearrange("(i r) s -> r i s", r=128),
                                in_=mpb[:, h].rearrange("r i f s -> r i (f s)"),
                            )
                        elif i % 2 == 1:
                            h = slice(i - 1, i + 1)
                            nc.vector.tensor_sub(
                                mpb[:, h, 1], px_t[:, h], mpb[:, h, 0])
                            nc.vector.tensor_mul(
                                mpb[:, h, 2], px_t[:, h], mpb[:, h, 0])
                            nc.sync.dma_start(
                                out=mp_d[b, 128 * (i - 1):128 * (i + 1)]
                                .rearrange("(i r) s -> r i s", r=128),
                                in_=mpb[:, h].rearrange("r i f s -> r i (f s)"),
                            )

    nc.compile()
    return nc


def _get_nc(reps: int = 1):
    key = f"nc{reps}"
    if key not in _CACHED:
        _CACHED[key] = _build(reps)
    return _CACHED[key]


def host_inputs(px, hx, p_mask, h_mask):
    """Full (all-core) input arrays keyed by DRAM tensor name.

    Leading dim of each array is NCORES x per-core leading dim; slicing
    it into NCORES equal chunks yields each core's in_map.
    """
    import ml_dtypes
    BF = ml_dtypes.bfloat16

    keep_h = ~np.asarray(h_mask)  # [B, S] True = keep
    keep_p = ~np.asarray(p_mask)
    # [r, b, j] per-partition exp bias: -SHIFT (keep) / MASK_BIAS (masked)
    def _bias(keep):
        k = keep.reshape(NCORES, NB, NBLK, 128).transpose(0, 3, 1, 2)
        return np.where(k, np.float32(-SHIFT), np.float32(MASK_BIAS)) \
            .astype(np.float32).reshape(NCORES * 128, NB, NBLK)
    pxf = np.asarray(px, dtype=np.float32)
    hxf = np.asarray(hx, dtype=np.float32)
    B = pxf.shape[0]
    inp = np.empty((B, 4, S, S), dtype=BF)
    inp[:, 0] = pxf.astype(BF)
    inp[:, 1] = hxf.astype(BF)
    inp[:, 2] = pxf.transpose(0, 2, 1).astype(BF)
    inp[:, 3] = hxf.transpose(0, 2, 1).astype(BF)
    return {
        "inp": inp,
        "bh": np.ascontiguousarray(_bias(keep_h)),
        "bp": np.ascontiguousarray(_bias(keep_p)),
        "_pxf": pxf,  # full-precision copies for the host splice
        "_hxf": hxf,
    }


def run_sharded(px, hx, p_mask, h_mask, **kw):
    """Shard over batch, run on 8 cores, return (results, BassKernelResults)."""
    from concourse.bass_utils import run_bass_kernel_spmd

    nc = _get_nc()
    full = host_inputs(px, hx, p_mask, h_mask)
    in_maps = []
    for c in range(NCORES):
        in_maps.append({
            "inp": full["inp"][NB * c:NB * (c + 1)],
            "bh": full["bh"][128 * c:128 * (c + 1)],
            "bp": full["bp"][128 * c:128 * (c + 1)],
        })
    res = run_bass_kernel_spmd(nc, in_maps, core_ids=list(range(NCORES)), **kw)
    # device ships [x_hat | diff | prod]; segment 0 of m_p/m_h is px/hx verbatim
    B = NCORES * NB
    mp = np.empty((B, S, 4 * S), np.float32)
    mh = np.empty((B, S, 4 * S), np.float32)
    mp[:, :, :S] = full["_pxf"]
    mh[:, :, :S] = full["_hxf"]
    mp[:, :, S:] = np.concatenate(
        [np.asarray(res.results[c]["mp"]) for c in range(NCORES)], axis=0)
    mh[:, :, S:] = np.concatenate(
        [np.asarray(res.results[c]["mh"]) for c in range(NCORES)], axis=0)
    return (mp, mh), res


def kernel(px, hx, p_mask, h_mask):
    (mp, mh), _ = run_sharded(px, hx, p_mask, h_mask)
    return mp, mh


# revision 81
# speedup vs baseline: 1.5602x; 1.5602x over previous
"""ESIM-style bidirectional cross-attention (LocalInterface) Bass kernel for TRN2.

Full inputs: px [32,512,512] f32, hx [32,512,512] f32, p_mask/h_mask [32,512] bool.
Data-parallel over batch: 8 NeuronCores x 4 batches each. Returns (m_p, m_h),
each [32,512,2048] f32.

v2 design (all-bf16 datapath; measured rel err 8.9e-3 vs the 2e-2 budget):

  The host ships px/hx in BOTH layouts as bf16 ([p,d] natural and [d,p]
  transposed, one stacked DRAM tensor, four 0.5MB DMAs/batch), so the
  device does ZERO input transposes: the e-matmul consumes the
  transposed copies directly and the value matmuls / output elementwise
  consume the natural copies. This removes 32 PE transposes and 8
  Act-engine PSUM evictions per batch vs v1, cutting the Act engine
  (the v1 bottleneck: 24 x ~720ns instructions/batch) down to the 8
  exps only it can do plus a balanced share of the evictions.

  e = px @ hx^T               bf16 matmuls, f32 PSUM accumulation
  u_aT[p,h] = exp(e + bias_p) Act exp during PSUM eviction; bias folds the
  u_bT[h,p] = exp(e^T+bias_h) constant shift -90 and the mask (-30090 ->
                              exp == 0) per partition, as in v1. u in bf16.
  e^T via PE f32r transpose (e must stay f32: |e| ~ 90, bf16 step ~0.5
  there would distort exp by ~60%; recomputing e^T as a second matmul
  pass measured WORSE on HW -- the 16 extra N=512 matmuls cost more
  than 16 transposes). The eT+exp stream is placed BETWEEN the two
  value-matmul groups so direction a's matmuls cover the e-eviction
  latency (sim: DMA engines 97.5% occupied with this order).
  px_hat = (u_bT^T @ hx) / s_b   value matmuls in bf16; s via 2-wide
  hx_hat = (u_aT^T @ px) / s_a   bf16 ones-column matmuls sharing the
                                 value matmul's stationary operand.
  Eviction work is balanced across the two capable engines: e copies
  split DVE/Act, direction-a hat scale on Act (Copy + per-partition
  scale AP), direction-b on DVE (tensor_scalar_mul), both writing
  straight into segment 0 of the output tile; DVE sub/mul fill
  segments 1/2 per 2-block half, and each half ships immediately
  (4 x 0.78MB output DMAs/batch) so the store overlaps compute.
  gpsimd does no elementwise work (its TensorTensor is 3.6x slower
  than DVE's) -- it only runs the mh output DMA ring + bias loads.
  The host prepends the verbatim px/hx segment and upcasts to f32
  after the gather.

Not adopted (measured): fp8-e4m3 prod segment (rel err 1.50e-2, no HW
speedup -- DVE fp8 stores + extra DMAs eat the byte savings); on-chip
pxT derivation (PE transposes lengthen the critical chain); host-
swizzled partition-contiguous DRAM layouts (defeats balance_dma_aps'
engine spray, 11us slower); merged 2MB input DMAs (serialize the SDMA
engines).

PSUM banks (8 total): e-matmul ring 2, eT ring 2, value ring 3 (the
deepest-cycled pool; swept 3/2/2 vs 2/3/2 vs 2/2/3 in the cost model,
value-ring depth wins), s-tiles 1. One-shot trims: batch 0's pxT/hxT
loads arrive in halves so the first e-matmuls start ~1us earlier, and
the final batch's last m_p half ships as two per-block DMAs on
alternating HWDGE rings to shorten the drain tail.

Per-core steady state (cost model): DMA 20.6MB -> 58.3us (the wall,
~97% occupied), PE ~47.6us (96 matmuls/batch), Act/DVE ~30us each,
gpsimd ~9us; one-shot 66.7us. Measured on HW: 46-49us/exec min
observed (M(16)/16), ~70-95us under tunnel contention.
"""

import numpy as np

NB = 4          # batches per core
NCORES = 8
S = 512         # P = H = D = 512
NBLK = 4        # 512 / 128
SHIFT = 90.0    # constant softmax shift (e ~ N(0, 512); see v1 analysis)
MASK_BIAS = -30090.0  # -SHIFT - 30000: exp underflows to exactly 0.0

_CACHED = {}


def _build(reps: int = 1):
    """Build the per-core Bass program.

    reps > 1 unrolls the whole per-core computation that many times
    (same inputs, same outputs) inside one NEFF; test.py uses this to
    measure steady-state per-execution time by differencing. The
    graded kernel() path always uses reps=1.
    """
    import concourse.tile as tile
    import concourse.mybir as mybir
    from concourse import bacc
    from concourse.masks import make_identity

    F32 = mybir.dt.float32
    F32R = mybir.dt.float32r
    BF16 = mybir.dt.bfloat16
    F8 = mybir.dt.float8e4
    EXP = mybir.ActivationFunctionType.Exp
    COPY = mybir.ActivationFunctionType.Copy

    nc = bacc.Bacc(None, target_bir_lowering=False)
    # stacked input tensor: [batch, tensor(px|hx|pxT|hxT), row, col]
    in_d = nc.dram_tensor("inp", [NB, 4, S, S], BF16, kind="ExternalInput")
    # exp biases, host-precomputed: [r, b, j] = -SHIFT if kept else MASK_BIAS
    bh_d = nc.dram_tensor("bh", [128, NB, NBLK], F32, kind="ExternalInput")
    bp_d = nc.dram_tensor("bp", [128, NB, NBLK], F32, kind="ExternalInput")
    mp_d = nc.dram_tensor("mp", [NB, S, 3 * S], BF16, kind="ExternalOutput")
    mh_d = nc.dram_tensor("mh", [NB, S, 3 * S], BF16, kind="ExternalOutput")

    with tile.TileContext(nc) as tc:
        with (
            tc.tile_pool(name="const", bufs=1) as const,
            tc.tile_pool(name="sbL", bufs=4) as sbL,
            tc.tile_pool(name="sbE", bufs=12) as sbE,
            tc.tile_pool(name="sbU", bufs=3) as sbU,
            tc.tile_pool(name="sbS", bufs=3) as sbS,
            tc.tile_pool(name="sbO", bufs=3) as sbO,
            tc.tile_pool(name="pe_p", bufs=2, space="PSUM") as pe_p,
            tc.tile_pool(name="pet_p", bufs=2, space="PSUM") as pet_p,
            tc.tile_pool(name="pv_p", bufs=3, space="PSUM") as pv_p,
            tc.tile_pool(name="ps_p", bufs=1, space="PSUM") as ps_p,
        ):
            ident = const.tile([128, 128], F32)
            make_identity(nc, ident)
            identr = const.tile([128, 128], F32R)
            nc.vector.tensor_copy(out=identr, in_=ident)
            # bf16 matmuls accept a 2-wide ones column for the s sums
            # (>=2-element contiguous PSUM dst requirement)
            ones_col = const.tile([128, 2], BF16)
            nc.vector.memset(ones_col, 1.0)
            # per-partition exp biases for every batch: one contiguous load
            bias_h = const.tile([128, NB, NBLK], F32)
            bias_p = const.tile([128, NB, NBLK], F32)
            nc.gpsimd.dma_start(out=bias_h, in_=bh_d[:, :, :])
            nc.gpsimd.dma_start(out=bias_p, in_=bp_d[:, :, :])

            for rep in range(reps):
                for b in range(NB):
                    # ---- loads: pxT/hxT first (feed the e-matmul), then
                    # the natural layouts (value matmuls / outputs) ----
                    ldT = sbL.tile([128, 2, NBLK, S], BF16, tag="ldT")
                    ldN = sbL.tile([128, 2, NBLK, S], BF16, tag="ldN")
                    if rep == 0 and b == 0:
                        # first batch: halved loads let the e-matmuls start
                        # on the first half ~1us earlier (one-shot ramp)
                        for t, eng in ((2, nc.sync), (3, nc.scalar)):
                            for hj in (0, 2):
                                eng.dma_start(
                                    out=ldT[:, t - 2, hj:hj + 2],
                                    in_=in_d[b, t, 128 * hj:128 * (hj + 2)]
                                    .rearrange("(i r) d -> r i d", r=128))
                    else:
                        nc.sync.dma_start(
                            out=ldT[:, 0],
                            in_=in_d[b, 2].rearrange("(i r) d -> r i d", r=128))
                        nc.scalar.dma_start(
                            out=ldT[:, 1],
                            in_=in_d[b, 3].rearrange("(i r) d -> r i d", r=128))
                    nc.sync.dma_start(
                        out=ldN[:, 0],
                        in_=in_d[b, 0].rearrange("(i r) d -> r i d", r=128))
                    nc.scalar.dma_start(
                        out=ldN[:, 1],
                        in_=in_d[b, 1].rearrange("(i r) d -> r i d", r=128))
                    px_t, hx_t = ldN[:, 0], ldN[:, 1]
                    pxT, hxT = ldT[:, 0], ldT[:, 1]

                    # ---- e = px @ hx^T [P,H]; u_aT = exp(e + bias_p) ----
                    # e PSUM->SBUF evictions split between DVE and Act.
                    e_sb = [sbE.tile([128, S], F32R, tag="e_sb",
                                     name=f"e_sb{rep}_{b}_{i}") for i in range(NBLK)]
                    u_aT = sbU.tile([128, NBLK, S], BF16, tag="u_aT")
                    for i in range(NBLK):
                        pe = pe_p.tile([128, S], F32, tag="pe")
                        for j in range(NBLK):
                            nc.tensor.matmul(
                                pe, pxT[:, j, 128 * i:128 * (i + 1)], hxT[:, j],
                                start=(j == 0), stop=(j == NBLK - 1),
                            )
                        nc.scalar.activation(
                            out=u_aT[:, i], in_=pe, func=EXP,
                            bias=bias_p[:, b, i:i + 1],
                        )
                        if i < 2:
                            nc.vector.tensor_copy(out=e_sb[i], in_=pe)
                        else:
                            nc.scalar.copy(out=e_sb[i], in_=pe)

                    r_t = sbS.tile([128, 2 * NBLK], F32, tag="r_t")

                    # ---- direction a (hx_hat, m_h): needs only u_aT ----
                    # hat eviction+scale on Act (Copy, per-partition scale);
                    # output assembled and shipped in 2-block halves so the
                    # DMA overlaps the remaining blocks' compute.
                    s_a = ps_p.tile([128, 2 * NBLK], F32, tag="sps")
                    mhb = sbO.tile([128, NBLK, 3, S], BF16, tag="mh_blk")
                    for j in range(NBLK):
                        pv = pv_p.tile([128, S], F32, tag="pv")
                        for i in range(NBLK):
                            nc.tensor.matmul(
                                pv, u_aT[:, i, 128 * j:128 * (j + 1)], px_t[:, i],
                                start=(i == 0), stop=(i == NBLK - 1),
                            )
                            nc.tensor.matmul(
                                s_a[:, 2 * j:2 * j + 2],
                                u_aT[:, i, 128 * j:128 * (j + 1)],
                                ones_col,
                                start=(i == 0), stop=(i == NBLK - 1),
                                skip_group_check=True,
                            )
                        nc.vector.reciprocal(
                            out=r_t[:, j:j + 1], in_=s_a[:, 2 * j:2 * j + 1])
                        nc.scalar.activation(
                            out=mhb[:, j, 0], in_=pv, func=COPY,
                            scale=r_t[:, j:j + 1])
                        if j % 2 == 1:
                            h = slice(j - 1, j + 1)
                            nc.vector.tensor_sub(
                                mhb[:, h, 1], hx_t[:, h], mhb[:, h, 0])
                            nc.vector.tensor_mul(
                                mhb[:, h, 2], hx_t[:, h], mhb[:, h, 0])
                            nc.gpsimd.dma_start(
                                out=mh_d[b, 128 * (j - 1):128 * (j + 1)]
                                .rearrange("(j r) s -> r j s", r=128),
                                in_=mhb[:, h].rearrange("r j f s -> r j (f s)"),
                            )

                    # ---- eT stream: PE f32r transpose of e (overlapped by
                    # direction a's value matmuls), exp -> u_bT ----
                    u_bT = sbU.tile([128, NBLK, S], BF16, tag="u_bT")
                    for j in range(NBLK):
                        pet = pet_p.tile([128, S], F32R, tag="pet")
                        for i in range(NBLK):
                            nc.tensor.matmul(
                                pet[:, 128 * i:128 * (i + 1)],
                                e_sb[i][:, 128 * j:128 * (j + 1)],
                                identr,
                                is_transpose=True,
                                start=(i == 0), stop=(i == NBLK - 1),
                                skip_group_check=True,
                            )
                        nc.scalar.activation(
                            out=u_bT[:, j], in_=pet, func=EXP,
                            bias=bias_h[:, b, j:j + 1],
                        )

                    # ---- direction b (px_hat, m_p): needs u_bT ----
                    # hat eviction+scale on DVE (tensor_scalar_mul) to balance.
                    s_b = ps_p.tile([128, 2 * NBLK], F32, tag="sps")
                    mpb = sbO.tile([128, NBLK, 3, S], BF16, tag="mp_blk")
                    for i in range(NBLK):
                        pv = pv_p.tile([128, S], F32, tag="pv")
                        for j in range(NBLK):
                            nc.tensor.matmul(
                                pv, u_bT[:, j, 128 * i:128 * (i + 1)], hx_t[:, j],
                                start=(j == 0), stop=(j == NBLK - 1),
                            )
                            nc.tensor.matmul(
                                s_b[:, 2 * i:2 * i + 2],
                                u_bT[:, j, 128 * i:128 * (i + 1)],
                                ones_col,
                                start=(j == 0), stop=(j == NBLK - 1),
                                skip_group_check=True,
                            )
                        nc.vector.reciprocal(
                            out=r_t[:, NBLK + i:NBLK + i + 1],
                            in_=s_b[:, 2 * i:2 * i + 1])
                        nc.vector.tensor_scalar_mul(
                            out=mpb[:, i, 0], in0=pv,
                            scalar1=r_t[:, NBLK + i:NBLK + i + 1])
                        fin = (rep == reps - 1 and b == NB - 1)
                        if fin and i >= 2:
                            # tail: per-block assembly + DMA on alternating
                            # HWDGE rings so the last store is only 0.39MB
                            h = slice(i, i + 1)
                            nc.vector.tensor_sub(
                                mpb[:, h, 1], px_t[:, h], mpb[:, h, 0])
                            nc.vector.tensor_mul(
                                mpb[:, h, 2], px_t[:, h], mpb[:, h, 0])
                            eng = nc.sync if i == 2 else nc.scalar
                            eng.dma_start(
                                out=mp_d[b, 128 * i:128 * (i + 1)]
                                .rearrange("(i r) s -> r i s", r=128),
                                in_=mpb[:, h].rearrange("r i f s -> r i (f s)"),
                            )
                        elif i % 2 == 1:
                            h = slice(i - 1, i + 1)
                            nc.vector.tensor_sub(
                                mpb[:, h, 1], px_t[:, h], mpb[:, h, 0])
                            nc.vector.tensor_mul(
                                mpb[:, h, 2], px_t[:, h], mpb[:, h, 0])
                            nc.sync.dma_start(
                                out=mp_d[b, 128 * (i - 1):128 * (i + 1)]
                                .rearrange("(i r) s -> r i s", r=128),
                                in_=mpb[:, h].rearrange("r i f s -> r i (f s)"),
                            )

    nc.compile()
    return nc


def _get_nc(reps: int = 1):
    key = f"nc{reps}"
    if key not in _CACHED:
        _CACHED[key] = _build(reps)
    return _CACHED[key]


def host_inputs(px, hx, p_mask, h_mask):
    """Full (all-core) input arrays keyed by DRAM tensor name.

    Leading dim of each array is NCORES x per-core leading dim; slicing
    it into NCORES equal chunks yields each core's in_map.
    """
    import ml_dtypes
    BF = ml_dtypes.bfloat16

    keep_h = ~np.asarray(h_mask)  # [B, S] True = keep
    keep_p = ~np.asarray(p_mask)
    # [r, b, j] per-partition exp bias: -SHIFT (keep) / MASK_BIAS (masked)
    def _bias(keep):
        k = keep.reshape(NCORES, NB, NBLK, 128).transpose(0, 3, 1, 2)
        return np.where(k, np.float32(-SHIFT), np.float32(MASK_BIAS)) \
            .astype(np.float32).reshape(NCORES * 128, NB, NBLK)
    pxf = np.asarray(px, dtype=np.float32)
    hxf = np.asarray(hx, dtype=np.float32)
    B = pxf.shape[0]
    inp = np.empty((B, 4, S, S), dtype=BF)
    inp[:, 0] = pxf.astype(BF)
    inp[:, 1] = hxf.astype(BF)
    inp[:, 2] = pxf.transpose(0, 2, 1).astype(BF)
    inp[:, 3] = hxf.transpose(0, 2, 1).astype(BF)
    return {
        "inp": inp,
        "bh": np.ascontiguousarray(_bias(keep_h)),
        "bp": np.ascontiguousarray(_bias(keep_p)),
        "_pxf": pxf,  # full-precision copies for the host splice
        "_hxf": hxf,
    }


def run_sharded(px, hx, p_mask, h_mask, **kw):
    """Shard over batch, run on 8 cores, return (results, BassKernelResults)."""
    from concourse.bass_utils import run_bass_kernel_spmd

    nc = _get_nc()
    full = host_inputs(px, hx, p_mask, h_mask)
    in_maps = []
    for c in range(NCORES):
        in_maps.append({
            "inp": full["inp"][NB * c:NB * (c + 1)],
            "bh": full["bh"][128 * c:128 * (c + 1)],
            "bp": full["bp"][128 * c:128 * (c + 1)],
        })
    res = run_bass_kernel_spmd(nc, in_maps, core_ids=list(range(NCORES)), **kw)
    # device ships [x_hat | diff | prod]; segment 0 of m_p/m_h is px/hx verbatim
    B = NCORES * NB
    mp = np.empty((B, S, 4 * S), np.float32)
    mh = np.empty((B, S, 4 * S), np.float32)
    mp[:, :, :S] = full["_pxf"]
    mh[:, :, :S] = full["_hxf"]
    mp[:, :, S:] = np.concatenate(
        [np.asarray(res.results[c]["mp"]) for c in range(NCORES)], axis=0)
    mh[:, :, S:] = np.concatenate(
        [np.asarray(res.results[c]["mh"]) for c in range(NCORES)], axis=0)
    return (mp, mh), res


def kernel(px, hx, p_mask, h_mask):
    (mp, mh), _ = run_sharded(px, hx, p_mask, h_mask)
    return mp, mh
